# revision 5
# baseline (speedup 1.0000x reference)
"""Trainium2 Bass kernel for nn_MultiHeadMixer.

Reference computation (B=4, S=2048, E=1024, H=16, D=64):
    xp = x @ inp_w.T + inp_b                      # (B,S,E)
    xh[b,h,d,s] = xp[b,s,h*D+d]
    y0[b,h,d,t] = sum_{s<=t} xh[b,h,d,s] * weight[h,t-s]   # causal Toeplitz
    y2 = y0 / cumsum(weight)[h,t] + bias[h,t]
    out[b,t,:] = (y2 reshaped to (E,)) @ out_w.T

Sharding (8 cores): core c = (batch-pair p=c//4, head-group g=c%4).
Each core computes batches {2p, 2p+1} for heads {4g..4g+3} (feature cols
F=[256g,256g+256)) and emits a full-(E) bf16 partial of out[b].T per
batch; host sums the four head-group partials per batch and transposes.

On-device phases (everything in the [feature/seq-transposed] domain):
  proj1:  xp[s, 256b+f] = sum_e xT_b[e,s] w1T[e,f]        (PE, K=e)
  mixer:  block-T form: out[tau,(b,d)] += Tblk[sig,tau]^T xp_i[sig,(b,d)]
          with Tblk = strip[:, 128*(j-i):+128]; both batches of one head
          fill the full 128-wide moving tensor, and the Delta=j-i loop
          reuses one stationary across a whole t-quad (ldweights amortized).
  epi:    y2T = y0T * invn_col + bias_col  (per-PARTITION scalars: the
          transposed domain makes the (h,t) normalization a [128,1] AP)
  transp: PE transposes y2T 64 cols per batch into head-pair PSUM banks,
          giving y2_b[feature, t] tiles for proj2.
  proj2:  outT_b[e',t] = sum_c w2T[c,e'] y2_b[c,t]        (PE, K=c)

PSUM discipline: every bank's first matmul after reuse is start=True
(clears the 2KiB zero-region's has_written bits); all other writes are
start=False, so first-touch overwrites and repeat-touch accumulates.
"""

import contextlib

import numpy as np
import ml_dtypes

import concourse.bass as bass
import concourse.bacc as bacc
import concourse.mybir as mybir
import concourse.tile as tile
from concourse.bass_utils import run_bass_kernel_spmd

B, S, E, H = 4, 2048, 1024, 16
D = E // H
N_CORES = 8
HPC = 4          # heads per core
FPC = 256        # feature cols per core
SB = S // 128    # 16 s/t-blocks
TQ = S // 512    # 4 t-quads

BF16 = mybir.dt.bfloat16
F32 = mybir.dt.float32
NPBF16 = ml_dtypes.bfloat16

_CACHED = {}


def emit_body(nc, tc, aps):
    xTb, w1T, b1x, Tp, invnT, biasT, w2T, ident_d, outX = aps
    AF = mybir.ActivationFunctionType
    ALU = mybir.AluOpType
    with (
        tc.tile_pool(name="w1", bufs=8) as w1_pool,
        tc.tile_pool(name="xt", bufs=16) as xt_pool,
        tc.tile_pool(name="cst", bufs=12) as cst_pool,
        tc.tile_pool(name="tp", bufs=HPC) as tp_pool,
        tc.tile_pool(name="w2", bufs=2) as w2_pool,
        tc.tile_pool(name="xp", bufs=SB) as xp_pool,
        tc.tile_pool(name="y2t", bufs=18) as y2t_pool,
        tc.tile_pool(name="y2s", bufs=6) as y2s_pool,
        tc.tile_pool(name="ost", bufs=4) as ost_pool,
        tc.tile_pool(name="psAP", bufs=2, space="PSUM") as psAP_pool,
        tc.tile_pool(name="psm", bufs=3, space="PSUM") as psm_pool,
        tc.tile_pool(name="psT", bufs=3, space="PSUM") as psT_pool,
    ):
        # ---- input loads, in consumption order ----
        w1_t = []
        for k in range(8):
            w = w1_pool.tile([128, FPC], BF16, tag="w1", name=f"w1_{k}")
            nc.sync.dma_start(w[:], w1T[128 * k:128 * (k + 1), :])
            w1_t.append(w)
        b1_t = cst_pool.tile([128, 2 * FPC], F32, tag="cst", name="b1_t")
        nc.sync.dma_start(b1_t[:], b1x[:])
        xt_t = {}
        for b in range(2):
            for k in range(8):
                t = xt_pool.tile([128, S], BF16, tag="xt", name=f"xt_{b}_{k}")
                nc.sync.dma_start(t[:], xTb[b][128 * k:128 * (k + 1), :])
                xt_t[(b, k)] = t
        tp_t = []
        for h in range(HPC):
            t = tp_pool.tile([128, S], BF16, tag="tp", name=f"tp_{h}")
            nc.sync.dma_start(t[:], Tp[h])
            tp_t.append(t)
        invn_t, bias_t = [], []
        for h in range(HPC):
            t = cst_pool.tile([128, SB], F32, tag="cst", name=f"invn_{h}")
            nc.sync.dma_start(t[:], invnT[h])
            invn_t.append(t)
            t = cst_pool.tile([128, SB], F32, tag="cst", name=f"bias_{h}")
            nc.sync.dma_start(t[:], biasT[h])
            bias_t.append(t)
        ident = cst_pool.tile([128, 128], BF16, tag="cst", name="ident")
        nc.sync.dma_start(ident[:], ident_d[:])
        w2_t = []
        for hp in range(2):
            t = w2_pool.tile([128, E], BF16, tag="w2", name=f"w2_{hp}")
            nc.sync.dma_start(t[:], w2T[hp])
            w2_t.append(t)

        # ---- proj1: xp[m] [128 s, 512] cols = [b(2) x f(256)] ----
        xp_t = []
        for m in range(SB):
            ps = psAP_pool.tile([128, 512], F32, tag="psAP", name=f"ps1_{m}")
            for b in range(2):
                for k in range(8):
                    nc.tensor.matmul(
                        ps[:, FPC * b:FPC * (b + 1)],
                        xt_t[(b, k)][:, 128 * m:128 * (m + 1)],
                        w1_t[k][:],
                        start=(b == 0 and k == 0),
                        stop=(b == 1 and k == 7),
                        skip_group_check=True,
                    )
            xp = xp_pool.tile([128, 512], BF16, tag="xp", name=f"xp_{m}")
            nc.vector.tensor_add(xp[:], ps[:], b1_t[:])
            xp_t.append(xp)

        def moving(i, h):
            a = xp_t[i][:].rearrange("p (b h d) -> p b h d", b=2, h=HPC)
            return a[:, :, h, :]

        # ---- per t-quad: mixer -> epilogue -> transposes -> proj2 ----
        for q in range(TQ):
            y2t = {}
            # mixer: bank (h, q) holds quarters j=4q..4q+3 ([tau, (b,d)])
            for h in range(HPC):
                ps = psm_pool.tile([128, 512], F32, tag="psm",
                                   name=f"psm_{h}_{q}")
                for dlt in range(4 * q + 4):
                    for j in range(max(4 * q, dlt), 4 * q + 4):
                        i = j - dlt
                        jj = j - 4 * q
                        nc.tensor.matmul(
                            ps[:, 128 * jj:128 * (jj + 1)],
                            tp_t[h][:, 128 * dlt:128 * (dlt + 1)],
                            moving(i, h),
                            start=(dlt == 0 and jj == 0),
                            stop=(dlt == 4 * q + 3 and jj == 3),
                            skip_group_check=True,
                        )
                for jj in range(4):
                    j = 4 * q + jj
                    y = y2t_pool.tile([128, 128], BF16, tag="y2t",
                                      name=f"y2t_{h}_{j}")
                    if h % 2 == 0:
                        nc.scalar.activation(
                            y[:], ps[:, 128 * jj:128 * (jj + 1)],
                            AF.Identity,
                            bias=bias_t[h][:, j:j + 1],
                            scale=invn_t[h][:, j:j + 1])
                    else:
                        nc.vector.tensor_scalar(
                            y[:], ps[:, 128 * jj:128 * (jj + 1)],
                            invn_t[h][:, j:j + 1],
                            bias_t[h][:, j:j + 1],
                            ALU.mult, ALU.add)
                    y2t[(h, jj)] = y

            # transposes: psT bank (hp, b) [128 c, 512 tau] (bf16)
            y2s = {}
            for hp in range(2):
                for b in range(2):
                    pst = psT_pool.tile([128, 1024], BF16, tag="psT",
                                        name=f"psT_{hp}_{b}_{q}")
                    for hh in range(2):
                        for jj in range(4):
                            nc.tensor.transpose(
                                pst[64 * hh:64 * (hh + 1),
                                    128 * jj:128 * (jj + 1)],
                                y2t[(2 * hp + hh, jj)][:, 64 * b:64 * (b + 1)],
                                ident[:])
                    ys = y2s_pool.tile([128, 512], BF16, tag="y2s",
                                       name=f"y2s_{b}_{hp}_{q}")
                    if (hp + b) % 2 == 0:
                        nc.vector.tensor_copy(ys[:], pst[:, 0:512])
                    else:
                        nc.scalar.copy(ys[:], pst[:, 0:512])
                    y2s[(b, hp)] = ys

            # proj2: out blocks (b, n) accumulate hp=0,1; K=128
            for n in range(8):
                pso = {}
                for hp in range(2):
                    for b in range(2):
                        if hp == 0:
                            pso[b] = psAP_pool.tile(
                                [128, 512], F32, tag="psAP",
                                name=f"ps2_{b}_{n}_{q}")
                        nc.tensor.matmul(
                            pso[b][:],
                            w2_t[hp][:, 128 * n:128 * (n + 1)],
                            y2s[(b, hp)][:],
                            start=(hp == 0),
                            stop=(hp == 1),
                            skip_group_check=True,
                        )
                for b in range(2):
                    ost = ost_pool.tile([128, 512], BF16, tag="ost",
                                        name=f"ost_{b}_{n}_{q}")
                    if n % 2 == 0:
                        nc.vector.tensor_copy(ost[:], pso[b][:])
                    else:
                        nc.scalar.copy(ost[:], pso[b][:])
                    nc.sync.dma_start(
                        outX[b][128 * n:128 * (n + 1),
                                512 * q:512 * (q + 1)],
                        ost[:],
                    )


def build_program(loop_n=None):
    nc = bacc.Bacc("TRN2", target_bir_lowering=False, debug=False,
                   num_devices=N_CORES)

    aps = (
        nc.dram_tensor("xTb", [2, E, S], BF16, kind="ExternalInput").ap(),
        nc.dram_tensor("w1T", [E, FPC], BF16, kind="ExternalInput").ap(),
        nc.dram_tensor("b1x", [128, 2 * FPC], F32, kind="ExternalInput").ap(),
        nc.dram_tensor("Tp", [HPC, 128, S], BF16, kind="ExternalInput").ap(),
        nc.dram_tensor("invnT", [HPC, 128, SB], F32, kind="ExternalInput").ap(),
        nc.dram_tensor("biasT", [HPC, 128, SB], F32, kind="ExternalInput").ap(),
        nc.dram_tensor("w2T", [2, 128, E], BF16, kind="ExternalInput").ap(),
        nc.dram_tensor("ident", [128, 128], BF16, kind="ExternalInput").ap(),
        nc.dram_tensor("outX", [2, E, S], BF16, kind="ExternalOutput").ap(),
    )

    with tile.TileContext(nc) as tc:
        with (tc.For_i(0, loop_n, 1) if loop_n else contextlib.nullcontext()):
            emit_body(nc, tc, aps)

    nc.compile()
    return nc


def host_prep(x, weight, bias, inp_w, inp_b, out_w):
    """Build the 8 per-core input maps (host-side shard + layout prep)."""
    x = np.asarray(x, np.float32)
    weight = np.asarray(weight, np.float32)
    bias = np.asarray(bias, np.float32)
    inp_w = np.asarray(inp_w, np.float32)
    inp_b = np.asarray(inp_b, np.float32)
    out_w = np.asarray(out_w, np.float32)

    invn = 1.0 / np.cumsum(weight, axis=1)
    ident = np.eye(128, dtype=NPBF16)

    xT_b = [np.ascontiguousarray(x[b].T).astype(NPBF16) for b in range(B)]
    xTb_p = [np.stack([xT_b[2 * p], xT_b[2 * p + 1]]) for p in range(2)]

    g_pack = []
    for g in range(4):
        cols = slice(FPC * g, FPC * (g + 1))
        w1T = np.ascontiguousarray(inp_w[cols, :].T).astype(NPBF16)
        b1row = np.concatenate([inp_b[cols], inp_b[cols]])
        b1x = np.broadcast_to(b1row, (128, 2 * FPC)).astype(np.float32).copy()
        w2T = np.stack([
            np.ascontiguousarray(
                out_w[:, FPC * g + 128 * hp:FPC * g + 128 * (hp + 1)].T)
            for hp in range(2)
        ]).astype(NPBF16)
        Tp = np.zeros((HPC, 128, S), np.float32)
        invnT = np.zeros((HPC, 128, SB), np.float32)
        biasT = np.zeros((HPC, 128, SB), np.float32)
        for hl in range(HPC):
            hgl = 4 * g + hl
            wrow = weight[hgl]
            wpad = np.concatenate([np.zeros(127, np.float32), wrow])
            Tp[hl] = np.lib.stride_tricks.as_strided(
                wpad[127:], shape=(128, S), strides=(-4, 4))
            invnT[hl] = invn[hgl].reshape(SB, 128).T
            biasT[hl] = bias[hgl].reshape(SB, 128).T
        g_pack.append(dict(w1T=w1T, b1x=b1x, w2T=w2T,
                           Tp=Tp.astype(NPBF16), invnT=invnT, biasT=biasT,
                           ident=ident))

    in_maps = []
    for c in range(N_CORES):
        p, g = c // 4, c % 4
        m = dict(g_pack[g])
        m["xTb"] = xTb_p[p]
        in_maps.append(m)
    return in_maps


def kernel(x, weight, bias, inp_w, inp_b, out_w):
    if "nc" not in _CACHED:
        _CACHED["nc"] = build_program()
    nc = _CACHED["nc"]

    in_maps = host_prep(x, weight, bias, inp_w, inp_b, out_w)
    res = run_bass_kernel_spmd(nc, in_maps, core_ids=list(range(N_CORES)))

    out = np.empty((B, S, E), np.float32)
    for b in range(B):
        p, bb = b // 2, b % 2
        acc = np.zeros((E, S), np.float32)
        for g in range(4):
            acc += np.asarray(res.results[4 * p + g]["outX"][bb],
                              dtype=np.float32)
        out[b] = acc.T
    return out


# revision 8
# speedup vs baseline: 1.0100x; 1.0100x over previous
"""Trainium2 Bass kernel for nn_MultiHeadMixer.

Reference computation (B=4, S=2048, E=1024, H=16, D=64):
    xp = x @ inp_w.T + inp_b                      # (B,S,E)
    xh[b,h,d,s] = xp[b,s,h*D+d]
    y0[b,h,d,t] = sum_{s<=t} xh[b,h,d,s] * weight[h,t-s]   # causal Toeplitz
    y2 = y0 / cumsum(weight)[h,t] + bias[h,t]
    out[b,t,:] = (y2 reshaped to (E,)) @ out_w.T

Sharding (8 cores): core c = (batch-pair p=c//4, head-group g=c%4).
Each core computes batches {2p, 2p+1} for heads {4g..4g+3} (feature cols
F=[256g,256g+256)) and emits a full-(E) bf16 partial of out[b].T per
batch; host sums the four head-group partials per batch and transposes.

On-device phases (PE at full 128 width everywhere):
  proj1:  xp[s, 256b+f] = sum_e xT_b[e,s] w1[e,f]           (K=e)
  mixer:  block-T: y0T[tau,(b,d)] += Tblk[sig,tau]^T xp_i[sig,(b,d)]
          with Tblk = strip[:, 128*(j-i):+128]; both batches of one head
          fill the 128-wide moving tensor; the Delta=j-i loop reuses one
          stationary across a t-quad.
  epi:    y2T = y0T*invn_col + bias_col (per-PARTITION [128,1] scalars)
  transp: PE transposes y2T 64 cols/batch into head-pair PSUM banks.
  proj2:  outT_b[e',t] = sum_c w2[c,e'] y2_b[c,t]           (K=c)

Schedule: the PE stream pipelines one quad deep (transposes/proj2 of
quad q run between mixer segments of q+1) so cross-engine epilogue and
copy latencies never stall the PE.  Loads ride the SP HWDGE ring, output
stores the Activation ring (no head-of-line blocking of next-iteration
prefetches); host pre-tiles xT so proj1's first tile needs only the
first xtt DMA per batch.

PSUM: every bank's first matmul after reuse is start=True (clears the
2KiB zero-region has_written bits); later first-touches overwrite and
repeat-touches accumulate.
"""

import contextlib

import numpy as np
import ml_dtypes

import concourse.bass as bass
import concourse.bacc as bacc
import concourse.mybir as mybir
import concourse.tile as tile
from concourse.bass_utils import run_bass_kernel_spmd

B, S, E, H = 4, 2048, 1024, 16
D = E // H
N_CORES = 8
HPC = 4          # heads per core
FPC = 256        # feature cols per core
SB = S // 128    # 16 s/t-blocks
TQ = S // 512    # 4 t-quads

BF16 = mybir.dt.bfloat16
F32 = mybir.dt.float32
NPBF16 = ml_dtypes.bfloat16

_CACHED = {}


def emit_body(nc, tc, aps):
    xtt_d, w1c_d, b1x_d, tpc_d, ivb_d, w2c_d, ident_d, outX = aps
    AF = mybir.ActivationFunctionType
    ALU = mybir.AluOpType
    with (
        tc.tile_pool(name="xtt", bufs=8) as xtt_pool,
        tc.tile_pool(name="tpc", bufs=1) as tpc_pool,
        tc.tile_pool(name="wc", bufs=2) as wc_pool,
        tc.tile_pool(name="cst", bufs=3) as cst_pool,
        tc.tile_pool(name="xp", bufs=SB) as xp_pool,
        tc.tile_pool(name="y2t", bufs=26) as y2t_pool,
        tc.tile_pool(name="y2s", bufs=8) as y2s_pool,
        tc.tile_pool(name="ost", bufs=16) as ost_pool,
        tc.tile_pool(name="psAP", bufs=2, space="PSUM") as psAP_pool,
        tc.tile_pool(name="psm", bufs=4, space="PSUM") as psm_pool,
        tc.tile_pool(name="psT", bufs=2, space="PSUM") as psT_pool,
    ):
        # ---- input loads (SP ring), in consumption order ----
        w1c = wc_pool.tile([128, 2048], BF16, tag="wc", name="w1c")
        nc.sync.dma_start(w1c[:], w1c_d[:])
        xtt_t = {}
        for mg in range(4):
            for b in range(2):
                t = xtt_pool.tile([128, 4096], BF16, tag="xtt",
                                  name=f"xtt_{b}_{mg}")
                nc.sync.dma_start(t[:], xtt_d[b][mg])
                xtt_t[(b, mg)] = t
        b1_t = cst_pool.tile([128, 2 * FPC], F32, tag="cst", name="b1_t")
        nc.sync.dma_start(b1_t[:], b1x_d[:])
        tpc = tpc_pool.tile([128, 4 * 2048], BF16, tag="tpc", name="tpc")
        nc.sync.dma_start(tpc[:], tpc_d[:])
        ivb = cst_pool.tile([128, 128], F32, tag="cst", name="ivb")
        nc.sync.dma_start(ivb[:], ivb_d[:])
        ident = cst_pool.tile([128, 128], BF16, tag="cst", name="ident")
        nc.sync.dma_start(ident[:], ident_d[:])
        w2c = wc_pool.tile([128, 2048], BF16, tag="wc", name="w2c")
        nc.sync.dma_start(w2c[:], w2c_d[:])

        def invn_col(h, j):
            return ivb[:, 16 * h + j:16 * h + j + 1]

        def bias_col(h, j):
            return ivb[:, 64 + 16 * h + j:64 + 16 * h + j + 1]

        # ---- proj1 ----
        xp_t = []
        for m in range(SB):
            mg, mm = m // 4, m % 4
            ps = psAP_pool.tile([128, 512], F32, tag="psAP", name=f"ps1_{m}")
            for b in range(2):
                for k in range(8):
                    nc.tensor.matmul(
                        ps[:, FPC * b:FPC * (b + 1)],
                        xtt_t[(b, mg)][:, 1024 * mm + 128 * k:
                                       1024 * mm + 128 * (k + 1)],
                        w1c[:, FPC * k:FPC * (k + 1)],
                        start=(b == 0 and k == 0),
                        stop=(b == 1 and k == 7),
                        skip_group_check=True,
                    )
            xp = xp_pool.tile([128, 512], BF16, tag="xp", name=f"xp_{m}")
            nc.vector.tensor_add(xp[:], ps[:], b1_t[:])
            xp_t.append(xp)

        def moving(i, h):
            a = xp_t[i][:].rearrange("p (b h d) -> p b h d", b=2, h=HPC)
            return a[:, :, h, :]

        y2t = {}
        y2s = {}

        def mix(q, hpair):
            for h in (2 * hpair, 2 * hpair + 1):
                ps = psm_pool.tile([128, 512], F32, tag="psm",
                                   name=f"psm_{h}_{q}")
                for dlt in range(4 * q + 4):
                    for j in range(max(4 * q, dlt), 4 * q + 4):
                        i = j - dlt
                        jj = j - 4 * q
                        nc.tensor.matmul(
                            ps[:, 128 * jj:128 * (jj + 1)],
                            tpc[:, 2048 * h + 128 * dlt:
                                2048 * h + 128 * (dlt + 1)],
                            moving(i, h),
                            start=(dlt == 0 and jj == 0),
                            stop=(dlt == 4 * q + 3 and jj == 3),
                            skip_group_check=True,
                        )
                for jj in range(4):
                    j = 4 * q + jj
                    y = y2t_pool.tile([128, 128], BF16, tag="y2t",
                                      name=f"y2t_{h}_{j}")
                    if h % 2 == 0:
                        nc.scalar.activation(
                            y[:], ps[:, 128 * jj:128 * (jj + 1)],
                            AF.Identity,
                            bias=bias_col(h, j), scale=invn_col(h, j))
                    else:
                        nc.vector.tensor_scalar(
                            y[:], ps[:, 128 * jj:128 * (jj + 1)],
                            invn_col(h, j), bias_col(h, j),
                            ALU.mult, ALU.add)
                    y2t[(h, jj, q)] = y

        def transp(q, hp):
            for b in range(2):
                pst = psT_pool.tile([128, 1024], BF16, tag="psT",
                                    name=f"psT_{hp}_{b}_{q}")
                for hh in range(2):
                    for jj in range(4):
                        nc.tensor.transpose(
                            pst[64 * hh:64 * (hh + 1),
                                128 * jj:128 * (jj + 1)],
                            y2t[(2 * hp + hh, jj, q)][:, 64 * b:64 * (b + 1)],
                            ident[:])
                ys = y2s_pool.tile([128, 512], BF16, tag="y2s",
                                   name=f"y2s_{b}_{hp}_{q}")
                if (hp + b) % 2 == 0:
                    nc.vector.tensor_copy(ys[:], pst[:, 0:512])
                else:
                    nc.scalar.copy(ys[:], pst[:, 0:512])
                y2s[(b, hp, q)] = ys

        ost_t = {}

        def proj2(q):
            for n in range(8):
                pso = {}
                for hp in range(2):
                    for b in range(2):
                        if hp == 0:
                            pso[b] = psAP_pool.tile(
                                [128, 512], F32, tag="psAP",
                                name=f"ps2_{b}_{n}_{q}")
                        nc.tensor.matmul(
                            pso[b][:],
                            w2c[:, 1024 * hp + 128 * n:
                                1024 * hp + 128 * (n + 1)],
                            y2s[(b, hp, q)][:],
                            start=(hp == 0),
                            stop=(hp == 1),
                            skip_group_check=True,
                        )
                for b in range(2):
                    if q == 0:
                        ost_t[(b, n)] = ost_pool.tile(
                            [128, 2048], BF16, tag="ost", name=f"ost_{b}_{n}")
                    ost = ost_t[(b, n)]
                    if n % 2 == 0:
                        nc.vector.tensor_copy(
                            ost[:, 512 * q:512 * (q + 1)], pso[b][:])
                    else:
                        nc.scalar.copy(
                            ost[:, 512 * q:512 * (q + 1)], pso[b][:])
                    if q == 3:
                        # stores ride the Activation HWDGE ring
                        nc.scalar.dma_start(
                            outX[b][128 * n:128 * (n + 1), :], ost[:])

        # ---- pipelined PE stream ----
        mix(0, 0)
        mix(0, 1)
        transp(0, 0)
        mix(1, 0)
        transp(0, 1)
        proj2(0)
        mix(1, 1)
        transp(1, 0)
        mix(2, 0)
        transp(1, 1)
        proj2(1)
        mix(2, 1)
        transp(2, 0)
        mix(3, 0)
        transp(2, 1)
        proj2(2)
        mix(3, 1)
        transp(3, 0)
        transp(3, 1)
        proj2(3)


def build_program(loop_n=None):
    nc = bacc.Bacc("TRN2", target_bir_lowering=False, debug=False,
                   num_devices=N_CORES)

    aps = (
        nc.dram_tensor("xtt", [2, 4, 128, 4096], BF16,
                       kind="ExternalInput").ap(),
        nc.dram_tensor("w1c", [128, 2048], BF16, kind="ExternalInput").ap(),
        nc.dram_tensor("b1x", [128, 2 * FPC], F32, kind="ExternalInput").ap(),
        nc.dram_tensor("tpc", [128, 4 * 2048], BF16,
                       kind="ExternalInput").ap(),
        nc.dram_tensor("ivb", [128, 128], F32, kind="ExternalInput").ap(),
        nc.dram_tensor("w2c", [128, 2048], BF16, kind="ExternalInput").ap(),
        nc.dram_tensor("ident", [128, 128], BF16, kind="ExternalInput").ap(),
        nc.dram_tensor("outX", [2, E, S], BF16, kind="ExternalOutput").ap(),
    )

    with tile.TileContext(nc) as tc:
        with (tc.For_i(0, loop_n, 1) if loop_n else contextlib.nullcontext()):
            emit_body(nc, tc, aps)

    nc.compile()
    return nc


def host_prep(x, weight, bias, inp_w, inp_b, out_w):
    """Build the 8 per-core input maps (host-side shard + layout prep)."""
    x = np.asarray(x, np.float32)
    weight = np.asarray(weight, np.float32)
    bias = np.asarray(bias, np.float32)
    inp_w = np.asarray(inp_w, np.float32)
    inp_b = np.asarray(inp_b, np.float32)
    out_w = np.asarray(out_w, np.float32)

    invn = 1.0 / np.cumsum(weight, axis=1)
    ident = np.eye(128, dtype=NPBF16)

    # xtt[b, mg][p, (mm,k,s)]: xT-block pretiling so proj1 tile m needs
    # only xtt[:, m//4]
    xtt_p = []
    for p in range(2):
        per_b = []
        for b in (2 * p, 2 * p + 1):
            A = np.ascontiguousarray(x[b].T)                 # [E, S]
            arr = A.reshape(8, 128, 16, 128).transpose(2, 1, 0, 3)  # [m,p,k,s]
            arr = arr.reshape(4, 4, 128, 8, 128).transpose(0, 2, 1, 3, 4)
            per_b.append(arr.reshape(4, 128, 4096))
        xtt_p.append(np.stack(per_b).astype(NPBF16))         # [2,4,128,4096]

    g_pack = []
    for g in range(4):
        cols = slice(FPC * g, FPC * (g + 1))
        w1c = (inp_w[cols, :].T.reshape(8, 128, FPC)
               .transpose(1, 0, 2).reshape(128, 2048)).astype(NPBF16)
        b1row = np.concatenate([inp_b[cols], inp_b[cols]])
        b1x = np.broadcast_to(b1row, (128, 2 * FPC)).astype(np.float32).copy()
        w2c = (out_w[:, cols].T.reshape(2, 128, E)
               .transpose(1, 0, 2).reshape(128, 2048)).astype(NPBF16)
        tpc = np.zeros((128, 4 * 2048), np.float32)
        ivb = np.zeros((128, 128), np.float32)
        for hl in range(HPC):
            hgl = 4 * g + hl
            wrow = weight[hgl]
            wpad = np.concatenate([np.zeros(127, np.float32), wrow])
            tpc[:, 2048 * hl:2048 * (hl + 1)] = np.lib.stride_tricks.as_strided(
                wpad[127:], shape=(128, S), strides=(-4, 4))
            ivb[:, 16 * hl:16 * (hl + 1)] = invn[hgl].reshape(SB, 128).T
            ivb[:, 64 + 16 * hl:64 + 16 * (hl + 1)] = \
                bias[hgl].reshape(SB, 128).T
        g_pack.append(dict(w1c=w1c, b1x=b1x, w2c=w2c,
                           tpc=tpc.astype(NPBF16), ivb=ivb, ident=ident))

    in_maps = []
    for c in range(N_CORES):
        p, g = c // 4, c % 4
        m = dict(g_pack[g])
        m["xtt"] = xtt_p[p]
        in_maps.append(m)
    return in_maps


def kernel(x, weight, bias, inp_w, inp_b, out_w):
    if "nc" not in _CACHED:
        _CACHED["nc"] = build_program()
    nc = _CACHED["nc"]

    in_maps = host_prep(x, weight, bias, inp_w, inp_b, out_w)
    res = run_bass_kernel_spmd(nc, in_maps, core_ids=list(range(N_CORES)))

    out = np.empty((B, S, E), np.float32)
    for b in range(B):
        p, bb = b // 2, b % 2
        acc = np.zeros((E, S), np.float32)
        for g in range(4):
            acc += np.asarray(res.results[4 * p + g]["outX"][bb],
                              dtype=np.float32)
        out[b] = acc.T
    return out
